# revision 18
# baseline (speedup 1.0000x reference)
"""Batch-sharded per-position linear: 8 cores x 512 batches, all 41 positions.

Pure batch split removes the old seq-half pad position (PE 64512 -> 61952
moving cols = 25.8us @ 2.4GHz) and the halo slices: edge taps drop out of
the matmul layout instead of multiplying shipped zeros. x ships as e3m4
of 2*inputs (power-of-2 scale, exact vs W), W stays bf16 moving; PSUM holds
2*z, relu gives 2*relu(z) <= 11.6 < e3m4 max 15.5, stored as e3m4 and the
host halves it (total l2 rel err ~0.0189 vs the 2e-2 budget).

Bytes per core: x 2.62MB + W 3.87MB + out 2.69MB = 9.18MB ~= 25.5us on the
shared 360GB/s DMA bus -- balanced against the 25.8us PE floor.

Schedule: 10 four-position groups + final 1-position group per batch
subtile (NBT=4). Group 0 accumulates per tile in 4 passes tracking W chunk
arrival. Loads split sync/HWDGE (W0a, x slices 2-6, W1..) and pool/SWDGE
(x slice 1 first -- jumps the bus ahead of the sync queue's second item --
then W0b, remaining x groups). ReLU+e3m4 downcast alternates DVE/ACT per
tile. The final position (41st) is computed as 2 two-subtile PSUM chains
so the stream ends on a tiny matmul->relu->store chain.
"""

import os
import sys

import numpy as np
import ml_dtypes

for _p in ("/opt/trn_rl_repo", "/root/.axon_site/_ro/trn_rl_repo"):
    if os.path.isdir(_p) and _p not in sys.path:
        sys.path.append(_p)

from contextlib import ExitStack

import concourse.mybir as mybir
import concourse.tile as tile
from concourse import bacc
from concourse.bass_utils import run_bass_kernel_spmd

S = 41
F = 128
WIN = 3
N_CORES = 8
B_FULL = 4096

NB = B_FULL // N_CORES    # 512 batches per core
NBT = NB // 128           # 4 batch sub-tiles
NS = S                    # 41 real x slices; slice i (1..41) = position i-1

# x load groups over slice indices 1..41: (slice0, nslices)
_XGROUPS = [(1, 6), (7, 1), (8, 6), (14, 7), (21, 7), (28, 7), (35, 7)]
_SLICE_MAP = {}
for _gi, (_s0, _n) in enumerate(_XGROUPS):
    for _k in range(_n):
        _SLICE_MAP[_s0 + _k] = (_gi, _k)

RELU_PARITY = 0
FINAL_RELU = 0
T0_GROUP = 6
G0_PASSES = [(0, 2), (2, 3), (3, 5)]
B2_RELU = 1

_nc_cache = {}


_GROUPDEF = [(j0, 4) for j0 in range(0, 40, 4)] + [(40, 1)]


def _layout2():
    """Matmul groups over positions 0..40. Entry (i, jmin, ncons):
    stationary x-slice i (slice i = input position i-1), consumers
    positions j in [jmin, jmin+ncons) with tap w = i - j. Slices are
    clamped to 1..41 so the zero halo taps at the global sequence edges
    are simply skipped."""
    out = []
    for j0, n in _GROUPDEF:
        ents = []
        for i in range(max(1, j0), min(S, j0 + n + 1) + 1):
            jmin = max(j0, i - 2)
            jmax = min(j0 + n - 1, i)
            if jmax >= jmin:
                ents.append((i, jmin, jmax - jmin + 1))
        out.append((j0, n, ents))
    return out


_LAYOUT = _layout2()
_WCOLS = [sum(nc_ * F for _, _, nc_ in ents) for _, _, ents in _LAYOUT]
_WTOT = sum(_WCOLS)
_NG = len(_LAYOUT)  # 11: ten 4-position groups + a 1-position unit (pos 40)


def _build(has_bias: bool):
    bf16 = mybir.dt.bfloat16
    f32 = mybir.dt.float32
    e3m4 = mybir.dt.float8e3
    nc = bacc.Bacc("TRN2", target_bir_lowering=False, debug=False)
    xT = nc.dram_tensor("xT", [NS, F, NB], e3m4, kind="ExternalInput").ap()
    Wg = nc.dram_tensor("Wg", [F, _WTOT], bf16, kind="ExternalInput").ap()
    bias = (
        nc.dram_tensor("bias", [1, S * F], bf16, kind="ExternalInput").ap()
        if has_bias
        else None
    )
    out = nc.dram_tensor("out", [NB, S, F], e3m4, kind="ExternalOutput").ap()

    def relu(dst, src, eng):
        if eng == 0:
            nc.vector.tensor_scalar_max(dst, src, 0.0)
        else:
            nc.scalar.activation(dst, src, mybir.ActivationFunctionType.Relu)

    with tile.TileContext(nc) as tc:
        with ExitStack() as ctx:
            xpool = ctx.enter_context(tc.tile_pool(name="xT", bufs=1))
            wpool = ctx.enter_context(tc.tile_pool(name="W", bufs=1))
            ppool = ctx.enter_context(tc.tile_pool(name="ps", bufs=8, space="PSUM"))
            opool = ctx.enter_context(tc.tile_pool(name="stage", bufs=_NG - 1))
            tpool = ctx.enter_context(tc.tile_pool(name="tail", bufs=1))

            # p-state warmup: pe_busy_start is pinned by the first matmul
            # execution (the entry Drain does not pin it; without this the
            # whole first 3us of the real stream runs at 0.65-1.2GHz). One
            # tiny memset + two [128,2]x[128,2] matmuls keep the entry
            # barrier delay minimal.
            wpool1 = ctx.enter_context(tc.tile_pool(name="warm", bufs=1))
            wsrc = wpool1.tile([128, 2], bf16, name="wsrc")
            nc.gpsimd.memset(wsrc[:], 0.0)
            ps_warm = ppool.tile([2, 2], f32, name="ps")
            for _ in range(2):
                nc.tensor.matmul(
                    ps_warm[:], lhsT=wsrc[:], rhs=wsrc[:], start=True, stop=True
                )

            xt = [xpool.tile([F, n * NB], e3m4, name=f"x{i}") for i, (_, n) in enumerate(_XGROUPS)]
            wt = [wpool.tile([F, _WCOLS[g]], bf16, name=f"w{g}") for g in range(_NG)]

            # first-matmul gate = W0a + slice 1 (the pool queue's first DMA
            # beats the sync queue's second onto the bus).
            x0 = xt[0][:].rearrange("k (s b) -> k s b", b=NB)
            half0 = sum(nc_ for _, _, nc_ in _LAYOUT[0][2][:3]) * F  # passA cols
            wc_of_g = [sum(_WCOLS[:g]) for g in range(_NG)]

            def xg_load(eng, gi):
                s0_, ns = _XGROUPS[gi]
                eng.dma_start(
                    xt[gi][:].rearrange("k (s b) -> k s b", b=NB)[:, :ns, :],
                    xT[s0_ - 1 : s0_ - 1 + ns].rearrange("s k b -> k s b"),
                )

            def w_load(eng, g):
                eng.dma_start(
                    wt[g][:], Wg[:, wc_of_g[g] : wc_of_g[g] + _WCOLS[g]]
                )

            # Pipe-fill plan: HWDGE generates one DMA per ~625ns, so the
            # sync queue issues few, chunked DMAs in exact consumption
            # order: W_g ships as piece A (ents 0-2; 0-1 for g0) and piece B
            # (rest), matching the two-pass matmul structure. pool/SWDGE
            # (parallel desc-gen) carries the bulk x stream in need order.
            def w_split(g):
                n_a = 2 if g == 0 else 3
                return sum(nc_ * F for _, _, nc_ in _LAYOUT[g][2][:n_a])

            def w_load_piece(eng, g, piece):
                sp = w_split(g)
                c0, c1 = (0, sp) if piece == 0 else (sp, _WCOLS[g])
                eng.dma_start(
                    wt[g][:, c0:c1],
                    Wg[:, wc_of_g[g] + c0 : wc_of_g[g] + c1],
                )

            # sync (HWDGE, 625ns gen cadence): s12, W0A, W0B, x4-6,
            # W1B, W2A, W10, W3A..W9A, W9B. pool (SWDGE, parallel ~1040ns
            # gen): x3, x7, W1A, x8-9, x10-13, W2B, x14-20, W3B, ... --
            # readiness times interleave so each piece lands on the bus just
            # ahead of its consumption deadline without jumping a more
            # urgent sync piece.
            nc.sync.dma_start(
                x0[:, 0:2, :], xT[0:2].rearrange("s k b -> k s b")
            )
            w_load_piece(nc.sync, 0, 0)
            w_load_piece(nc.sync, 0, 1)
            nc.sync.dma_start(
                x0[:, 3:6, :], xT[3:6].rearrange("s k b -> k s b")
            )
            w_load_piece(nc.sync, 1, 1)
            w_load_piece(nc.sync, 2, 0)
            w_load_piece(nc.sync, 3, 0)
            w_load_piece(nc.sync, 3, 1)
            w_load(nc.sync, _NG - 1)
            for g in range(4, _NG - 1):
                w_load_piece(nc.sync, g, 0)
                w_load_piece(nc.sync, g, 1)

            nc.gpsimd.dma_start(
                x0[:, 2:3, :], xT[2:3].rearrange("s k b -> k s b")
            )
            xg_load(nc.gpsimd, 1)          # x7
            w_load_piece(nc.gpsimd, 1, 0)  # W1A
            x2_ = xt[2][:].rearrange("k (s b) -> k s b", b=NB)
            nc.gpsimd.dma_start(
                x2_[:, 0:2, :], xT[7:9].rearrange("s k b -> k s b")
            )
            nc.gpsimd.dma_start(
                x2_[:, 2:6, :], xT[9:13].rearrange("s k b -> k s b")
            )
            w_load_piece(nc.gpsimd, 2, 1)
            xg_load(nc.gpsimd, 3)          # x14-20
            xg_load(nc.gpsimd, 4)          # x21-27
            xg_load(nc.gpsimd, 5)          # x28-34
            xg_load(nc.gpsimd, 6)          # x35-41

            if has_bias:
                bpool = ctx.enter_context(tc.tile_pool(name="bias", bufs=1))
                bias_sb = bpool.tile([1, S * F], bf16)
                nc.sync.dma_start(bias_sb[:], bias[:])
                ones = bpool.tile([1, F], bf16)
                nc.vector.memset(ones[:], 1.0)

            out_r = out.rearrange("(t p) s f -> p t s f", p=128)

            def lhsT(si, bt):
                gi, sub = _SLICE_MAP[si]
                return xt[gi][:, sub * NB + bt * 128 : sub * NB + (bt + 1) * 128]

            s5, _, ents5 = _LAYOUT[_NG - 1]
            w5_of, wc5 = [], 0
            for si, jmin, ncons in ents5:
                w5_of.append(wc5)
                wc5 += ncons * F
            n_mm5 = len(ents5) + (1 if has_bias else 0)

            def emit_p40():
                """Position 40 for all four subtiles as one 1-bank PSUM
                tile, emitted mid-stream where bus and relu slack is free,
                so the stream end stays uniform 4-position groups."""
                ps = ppool.tile([128, NBT * F], f32, name="ps")
                for bt in range(NBT):
                    for j, (si, _, _) in enumerate(ents5):
                        nc.tensor.matmul(
                            ps[:, bt * F : (bt + 1) * F],
                            lhsT=lhsT(si, bt),
                            rhs=wt[_NG - 1][:, w5_of[j] : w5_of[j] + F],
                            start=(j == 0),
                            stop=(j == n_mm5 - 1),
                        )
                    if has_bias:
                        nc.tensor.matmul(
                            ps[:, bt * F : (bt + 1) * F],
                            lhsT=ones[:],
                            rhs=bias_sb[:, s5 * F : (s5 + 1) * F],
                            start=False,
                            stop=True,
                        )
                ts = tpool.tile([128, NBT * F], e3m4, name="tstage")
                relu(ts[:], ps[:], eng=1)
                nc.sync.dma_start(
                    out_r[:, :, s5 : s5 + 1, :],
                    ts[:].rearrange("p (t s f) -> p t s f", t=NBT, s=1),
                )

            # --- groups 0..9: four positions per PSUM tile ---
            for g in range(_NG - 1):
                s0, npos, ents = _LAYOUT[g]
                stage = opool.tile([128, NBT * npos * F], e3m4, tag="stage")
                stage_c = stage[:].rearrange("p (t c) -> p t c", t=NBT)
                n_mm = len(ents) + (1 if has_bias else 0)

                if g == 0:
                    passes = G0_PASSES
                elif g < 7:
                    # two passes per group: W piece B only needed ~1.3us
                    # after the group starts, halving JIT load pressure
                    passes = [(0, 3), (3, len(ents))]
                else:
                    # late groups: W is long-delivered; single pass keeps
                    # each subtile's relu right behind its matmuls so the
                    # end-of-stream stores spread out instead of piling up
                    passes = [(0, len(ents))]
                tiles = []
                wcol_of = []
                wc = 0
                for si, jmin, ncons in ents:
                    wcol_of.append(wc)
                    wc += ncons * F
                for pi, (e0, e1) in enumerate(passes):
                    for bt in range(NBT):
                        if pi == 0:
                            ps = ppool.tile([128, npos * F], f32)
                            tiles.append(ps)
                        else:
                            ps = tiles[bt]
                        for j in range(e0, e1):
                            si, jmin, ncons = ents[j]
                            c0 = (jmin - s0) * F
                            nc.tensor.matmul(
                                ps[:, c0 : c0 + ncons * F],
                                lhsT=lhsT(si, bt),
                                rhs=wt[g][:, wcol_of[j] : wcol_of[j] + ncons * F],
                                start=(j == 0),
                                stop=(j == n_mm - 1),
                            )
                        if has_bias and e1 == len(ents):
                            nc.tensor.matmul(
                                ps[:, : npos * F],
                                lhsT=ones[:],
                                rhs=bias_sb[:, s0 * F : (s0 + npos) * F],
                                start=False,
                                stop=True,
                            )
                        if e1 == len(ents):
                            # stream-ending tile: split relu across DVE+ACT
                            # so the one trailing store chain starts ASAP
                            final_tile = g == _NG - 2 and bt == NBT - 1
                            pv = ps[:].rearrange("p (h c) -> p h c", h=1)
                            if final_tile:
                                hw_ = (npos * F) // 2
                                sc = stage_c[:, bt : bt + 1, :]
                                relu(sc[:, :, :hw_], pv[:, :, :hw_], eng=0)
                                relu(
                                    sc[:, :, hw_ : npos * F],
                                    pv[:, :, hw_ : npos * F],
                                    eng=1,
                                )
                            else:
                                feng = (bt + RELU_PARITY) % 2
                                if g == _NG - 2 and bt == NBT - 2:
                                    feng = B2_RELU
                                relu(
                                    stage_c[:, bt : bt + 1, : npos * F],
                                    pv[:, :, : npos * F],
                                    eng=feng,
                                )
                stage_v = stage[:].rearrange("p (t s f) -> p t s f", t=NBT, f=F)
                if g == _NG - 2:
                    # last group: bt0-1 merged, bt2, then the single trailing
                    # bt3 store right behind its split relu
                    nc.sync.dma_start(
                        out_r[:, 0:2, s0 : s0 + npos, :],
                        stage_v[:, 0:2, :npos, :],
                    )
                    nc.scalar.dma_start(
                        out_r[:, 2:3, s0 : s0 + npos, :],
                        stage_v[:, 2:3, :npos, :],
                    )
                    nc.sync.dma_start(
                        out_r[:, 3:4, s0 : s0 + npos, :],
                        stage_v[:, 3:4, :npos, :],
                    )
                else:
                    nc.sync.dma_start(
                        out_r[:, :, s0 : s0 + npos, :],
                        stage_v[:, :, :npos, :],
                    )
                if g == T0_GROUP:
                    emit_p40()

    nc.compile()
    return nc


def _get_nc(has_bias: bool):
    if has_bias not in _nc_cache:
        _nc_cache[has_bias] = _build(has_bias)
    return _nc_cache[has_bias]


def _prep_in_maps(inputs: np.ndarray, W: np.ndarray, b: np.ndarray):
    has_bias = bool(np.any(b))
    xb = (2.0 * inputs).astype(ml_dtypes.float8_e3m4)
    Wb = W.astype(ml_dtypes.bfloat16)
    blocks = []
    for s0, npos, ents in _LAYOUT:
        for si, jmin, ncons in ents:
            for j in range(jmin, jmin + ncons):
                w = si - 1 - j + 1  # tap index: slice si = position si-1
                blocks.append(Wb[j, w * F : (w + 1) * F, :])
    Wg = np.ascontiguousarray(np.concatenate(blocks, axis=1))
    assert Wg.shape == (F, _WTOT), Wg.shape
    if has_bias:
        bias_full = np.ascontiguousarray(
            b.astype(ml_dtypes.bfloat16).reshape(1, S * F)
        )

    in_maps = []
    for c in range(N_CORES):
        xs = np.ascontiguousarray(
            xb[c * NB : (c + 1) * NB, :, :].transpose(1, 2, 0)
        )
        m = {"xT": xs, "Wg": Wg}
        if has_bias:
            m["bias"] = bias_full
        in_maps.append(m)
    return in_maps, has_bias


def kernel(inputs: np.ndarray, W: np.ndarray, b: np.ndarray) -> np.ndarray:
    inputs = np.asarray(inputs)
    W = np.asarray(W)
    b = np.asarray(b)
    assert inputs.shape == (B_FULL, S, F), inputs.shape
    in_maps, has_bias = _prep_in_maps(inputs, W, b)
    nc = _get_nc(has_bias)
    res = run_bass_kernel_spmd(nc, in_maps, list(range(N_CORES)))
    out = np.empty((B_FULL, S, F), np.float32)
    for c in range(N_CORES):
        out[c * NB : (c + 1) * NB] = 0.5 * res.results[c]["out"].astype(
            np.float32
        )
    return out
